# revision 11
# baseline (speedup 1.0000x reference)
"""Crystalformer multihead attention (per-crystal dense blocks) on 8 TRN2 cores.

Problem: 64 crystals x 64 atoms; edges form a dense 64x64 block per crystal, so
the segment-softmax attention is per-crystal dense attention with a per-edge
additive logit bias (attn_weights) and a per-edge additive value term (values,
512MB fp32 -- the dominant HBM stream).

Strategy (data-parallel over crystals, 8 crystals/core), v2:
  * `values` streams in fp8 (e4m3, TRN max-normal 240) -- 4x less HBM traffic
    than fp32. The host bakes -ln(32) into attn_weights so e = exp(S') = p/32
    maxes at ~159 < 240: the fp8 softmax weights and the normalization
    (den' = den/32) absorb the scale exactly; no in-kernel rescaling.
  * term2 (sum_j w_ij * values_ij) keeps the values tile as the stationary
    operand (matmul output partitions are hard-wired to stationary columns,
    so the i-index must live on psum columns), but packs TWO heads per
    stationary: 128 fp8 columns triggers the compiler's Fast Weight Load
    (32b/cycle weight reads) -- 2-4x faster LDWEIGHTS than the fp32 baseline
    per byte, 8x-16x per value. The moving operand is a 4-column block-diag
    fp8 weight slice; output [128=(h2,d), 4] quadrants interleave two heads,
    unscrambled for free in the psum->sbuf merge pass.
  * q/k arrive pre-transposed (q pre-scaled by 1/sqrt(dh)) in bf16 -- the
    kernel has no input transposes; aw/v/out are bf16 too.
  * The upper 64 partitions of S hold scores shifted by one query (i+1), so
    exp writes the block-diagonal fp8 weights for even/odd queries with just
    two strided ACT ops. Weight-tile zero slots are memset once at init.
"""

import sys
from contextlib import ExitStack

import numpy as np

sys.path.insert(0, "/opt/trn_rl_repo")

import ml_dtypes  # noqa: E402

import concourse.bacc as bacc  # noqa: E402
import concourse.bass as bass  # noqa: E402
import concourse.tile as tile  # noqa: E402
from concourse import masks, mybir  # noqa: E402
from concourse.bass_utils import run_bass_kernel_spmd  # noqa: E402

G, NATOMS, H, DH = 64, 64, 8, 64
CORES = 8
GPC = G // CORES                 # crystals per core
ROWS = GPC * NATOMS              # atom rows per core
EROWS = GPC * NATOMS * NATOMS    # edge rows per core
F = H * DH                       # flattened (head, dh) = 512
LN32 = float(np.log(32.0))

_NC_CACHE = {}


def build_nc():
    f32 = mybir.dt.float32
    bf16 = mybir.dt.bfloat16
    fp8 = mybir.dt.float8e4

    nc = bacc.Bacc()
    qT_d = nc.declare_dram_parameter("qT", [ROWS, F], bf16, isOutput=False)
    kT_d = nc.declare_dram_parameter("kT", [ROWS, F], bf16, isOutput=False)
    v_d = nc.declare_dram_parameter("v", [ROWS, F], bf16, isOutput=False)
    aw_d = nc.declare_dram_parameter("aw", [ROWS, F], bf16, isOutput=False)
    # fp8 values, 4-segment packed: row (t4, ki), col (g2, seg, h, d);
    # 4 DMA tiles of [128, 4096] per crystal
    vals_d = nc.declare_dram_parameter("vals", [GPC * 512, 4096], fp8,
                                       isOutput=False)
    out_d = nc.declare_dram_parameter("out", [ROWS, F], bf16, isOutput=True)

    PS = bass.MemorySpace.PSUM
    with tile.TileContext(nc) as tc, ExitStack() as ctx:
        const = ctx.enter_context(tc.tile_pool(name="const", bufs=1))
        io = ctx.enter_context(tc.tile_pool(name="io", bufs=2))
        work = ctx.enter_context(tc.tile_pool(name="work", bufs=2))
        valsp = ctx.enter_context(tc.tile_pool(name="valsp", bufs=8))
        osb = ctx.enter_context(tc.tile_pool(name="osb", bufs=2))
        sps = ctx.enter_context(tc.tile_pool(name="sps", bufs=2, space=PS))
        ups = ctx.enter_context(tc.tile_pool(name="ups", bufs=2, space=PS))
        o1ps = ctx.enter_context(tc.tile_pool(name="o1ps", bufs=2, space=PS))
        fps = ctx.enter_context(tc.tile_pool(name="fps", bufs=2, space=PS))

        ident = const.tile([128, 128], bf16)
        masks.make_identity(nc, ident[:])
        ones = const.tile([64, 1], bf16)
        nc.vector.memset(ones[:], 1.0)
        # pz (block-diag fp8 weights, layout [p, (h, i)]; nonzero iff
        # p//64 == i%2): zero slots are written once here and never again --
        # the per-crystal ACT ops only fill the diagonal slots.
        pz_tiles = [const.tile([128, F], fp8, name=f"pz{i}", tag=f"pz{i}")
                    for i in range(2)]
        for t in pz_tiles:
            nc.vector.memset(t[:], 0.0)

        for ci in range(GPC):
            r0 = ci * NATOMS
            pz = pz_tiles[ci % 2]

            o_ps = fps.tile([64, F], bf16, tag="o_ps")
            # o1 rows 64:128: cols 0:8 hold den, col 16 is the absorber sink
            o1 = o1ps.tile([128, F], f32, tag="o1")

            def absorb(src_ap, sink=o1):
                # tiny matmul that pulls a producer's semaphore tick into
                # PE's vector clock (keeps hot matmuls at <=1 wait)
                nc.tensor.matmul(sink[64:65, 16:17], lhsT=src_ap[:, 0:1],
                                 rhs=src_ap[:, 0:1],
                                 start=True, stop=True, skip_group_check=True)

            if ci == 0:
                absorb(pz_tiles[0][0:64, :])
                absorb(pz_tiles[1][0:64, :])
                absorb(ident[0:64, :])

            qT_t = io.tile([64, F], bf16, tag="qT")
            nc.scalar.dma_start(qT_t[:], qT_d[r0:r0 + 64, :])
            kT_t = io.tile([64, F], bf16, tag="kT")
            nc.scalar.dma_start(kT_t[:], kT_d[r0:r0 + 64, :])
            v_t = io.tile([64, F], bf16, tag="v")
            nc.gpsimd.dma_start(v_t[:], v_d[r0:r0 + 64, :])
            aw_t = io.tile([64, F], bf16, tag="aw")
            nc.gpsimd.dma_start(aw_t[:], aw_d[r0:r0 + 64, :])

            vts = []
            for tt in range(4):
                vt = valsp.tile([128, 4096], fp8, tag="vt")
                nc.sync.dma_start(
                    vt[:], vals_d[(ci * 4 + tt) * 128:(ci * 4 + tt + 1) * 128, :])
                vts.append(vt)

            # absorb q/aw/v DMA ticks so S/bias/term1 matmuls carry <=1 wait
            absorb(qT_t[0:64, :])
            absorb(aw_t[0:64, :])
            absorb(v_t[0:64, :])

            # S[j, (h, i)] = K.T Qs + aw^T; upper partitions hold the i+1
            # shift (odd-query weight slots). One start (first matmul), one
            # stop (last lower-bias write -- the tick exp and the pz ACT ops
            # wait on).
            S = sps.tile([128, F], f32, tag="S")
            awv = aw_t[:].rearrange("p (j h) -> p h j", h=H)
            for h in range(H):
                hs = slice(h * DH, (h + 1) * DH)
                nc.tensor.matmul(S[0:64, hs], lhsT=kT_t[:, hs], rhs=qT_t[:, hs],
                                 start=(h == 0), stop=False,
                                 skip_group_check=True)
                ncols = DH if h < H - 1 else DH - 1
                nc.tensor.matmul(S[64:128, h * DH:h * DH + ncols],
                                 lhsT=kT_t[:, hs],
                                 rhs=qT_t[:, h * DH + 1:h * DH + 1 + ncols],
                                 start=(h == 0), stop=False,
                                 skip_group_check=True)
            for h in range(H):
                hs = slice(h * DH, (h + 1) * DH)
                nc.tensor.matmul(S[64:128, hs],
                                 lhsT=awv[:, h, :], rhs=ident[0:64, 1:65],
                                 start=False, stop=False,
                                 skip_group_check=True)
            for h in range(H):
                hs = slice(h * DH, (h + 1) * DH)
                nc.tensor.matmul(S[0:64, hs], lhsT=awv[:, h, :],
                                 rhs=ident[0:64, 0:64],
                                 start=False, stop=(h == H - 1),
                                 skip_group_check=True)

            # e = exp(S'): p2 (bf16) feeds term1 + den; two strided ACT ops
            # write the block-diag fp8 slots of pz (lower half: even queries,
            # upper half: odd queries via the S shift)
            p2 = work.tile([64, F], bf16, tag="p2")
            nc.scalar.activation(p2[:], S[0:64, :],
                                 mybir.ActivationFunctionType.Exp)
            pzS = pz[:].rearrange("p (h t s) -> p h t s", t=32, s=2)
            Sv = S[:].rearrange("p (h t s) -> p h t s", t=32, s=2)
            nc.scalar.activation(pzS[0:64, :, :, 0], Sv[0:64, :, :, 0],
                                 mybir.ActivationFunctionType.Exp)
            nc.scalar.activation(pzS[64:128, :, :, 1], Sv[64:128, :, :, 0],
                                 mybir.ActivationFunctionType.Exp)

            # absorb this crystal's pz ACT tick (c-2's DVE merge tick is
            # already in PE's clock via c-1's transposes reading tmpT)
            tmpT = work.tile([128, 256], bf16, tag="tmpT")
            absorb(pz[64:128, F - 1:F])

            # term2, transposed: stationary = [128 edges, 128=(2 heads, d)]
            # fp8 values slice (FWL), moving = 4 block-diag weight columns
            # (2 heads x 2 queries). outT column = (hp, t, h2m, s); valid
            # quadrants have h2(row) == h2m.
            outT = ups.tile([128, F], f32, tag="outT")
            outTv = outT[:].rearrange("p (hp t m s) -> p hp t m s",
                                      hp=4, t=32, m=2, s=2)
            first = True
            for tt in range(4):
                vtv = vts[tt][:].rearrange("p (g s f) -> p g s f",
                                           g=2, s=4, f=F)
                for g2 in range(2):
                    for sg in range(4):
                        t = (tt * 2 + g2) * 4 + sg
                        for hp in range(4):
                            nc.tensor.matmul(
                                outTv[:, hp, t, :, :],
                                lhsT=vtv[:, g2, sg, hp * 128:(hp + 1) * 128],
                                rhs=pzS[:, 2 * hp:2 * hp + 2, t, :],
                                start=first, stop=(t == 31 and hp == 3),
                                skip_group_check=True)
                            first = False

            # den + term1 share a psum bank: den at rows 64:128 (cols 0:8),
            # term1 (transposed: [d, (h, i)]) at rows 0:64
            for h in range(H):
                hs = slice(h * DH, (h + 1) * DH)
                nc.tensor.matmul(o1[64:128, h:h + 1], lhsT=p2[:, hs],
                                 rhs=ones[:], start=(h == 0), stop=False,
                                 skip_group_check=True)
            vv = v_t[:].rearrange("p (h d) -> p h d", d=DH)
            for h in range(H):
                hs = slice(h * DH, (h + 1) * DH)
                nc.tensor.matmul(o1[0:64, hs], lhsT=vv[:, h, :],
                                 rhs=p2[:, hs], start=False,
                                 stop=(h == H - 1), skip_group_check=True)

            # merge term2 quadrants + term1 into tmpT[(h2, d), (hp, i)] (two
            # DVE adds, one per h2 row-half), then transpose per head
            o1v = o1[:].rearrange("p (hp h2 t s) -> p hp h2 t s",
                                  hp=4, h2=2, t=32, s=2)
            tv = tmpT[:].rearrange("p (hp t s) -> p hp t s", hp=4, t=32, s=2)
            for h2 in range(2):
                nc.vector.tensor_tensor(
                    tv[h2 * 64:(h2 + 1) * 64, :, :, :],
                    outTv[h2 * 64:(h2 + 1) * 64, :, :, h2, :],
                    o1v[0:64, :, h2, :, :],
                    op=mybir.AluOpType.add)

            rden = work.tile([64, 8], f32, tag="rden")
            nc.vector.reciprocal(rden[:], o1[64:128, 0:8])

            for h in range(H):
                h2 = h % 2
                nc.tensor.transpose(
                    o_ps[0:64, h * DH:(h + 1) * DH],
                    tmpT[h2 * 64:h2 * 64 + 64,
                         (h // 2) * 64:(h // 2) * 64 + 64],
                    ident[h2 * 64:h2 * 64 + 64, h2 * 64:h2 * 64 + 64])

            o_sb = osb.tile([64, F], bf16, tag="o")
            nc.vector.tensor_tensor(
                o_sb[:].rearrange("p (h d) -> p h d", d=DH),
                o_ps[0:64, :].rearrange("p (h d) -> p h d", d=DH),
                rden[:].unsqueeze(2).broadcast_to([64, H, DH]),
                op=mybir.AluOpType.mult)
            nc.gpsimd.dma_start(out_d[r0:r0 + 64, :], o_sb[:])
    if not nc.is_finalized():
        nc.finalize()
    return nc


def _get_nc():
    if "nc" not in _NC_CACHE:
        _NC_CACHE["nc"] = build_nc()
    return _NC_CACHE["nc"]


def _edges_are_dense_blocks(e):
    base = np.arange(G, dtype=np.int64)[:, None, None] * NATOMS
    idx = np.arange(NATOMS, dtype=np.int64)
    e0 = np.broadcast_to(base + idx[None, :, None], (G, NATOMS, NATOMS)).reshape(-1)
    e1 = np.broadcast_to(base + idx[None, None, :], (G, NATOMS, NATOMS)).reshape(-1)
    return np.array_equal(e[0], e0) and np.array_equal(e[1], e1)


def _numpy_fallback(q, k, v, attn_weights, values, edges):
    # general (arbitrary-edges) segment-softmax path; slow but exact
    N = q.shape[0]
    e0, e1 = edges[0].astype(np.int64), edges[1].astype(np.int64)
    a = np.einsum("mhd,mhd->mh", q[e0] / np.sqrt(DH), k[e1]) + attn_weights
    m = np.full((N, H), -np.inf, np.float32)
    np.maximum.at(m, e0, a)
    p = np.exp(a - m[e0])
    den = np.zeros((N, H), np.float32)
    np.add.at(den, e0, p)
    w = p / den[e0]
    out = np.zeros_like(q)
    np.add.at(out, e0, w[:, :, None] * (v[e1] + values))
    return out


def make_in_maps(q, k, v, attn_weights, values):
    bf = ml_dtypes.bfloat16
    f8 = ml_dtypes.float8_e4m3  # TRN FP8_EXP4 semantics (max normal 240)

    in_maps = []
    for c in range(CORES):
        ra = slice(c * ROWS, (c + 1) * ROWS)
        re = slice(c * EROWS, (c + 1) * EROWS)
        # qT/kT: row (ci, d), col (h, i); q pre-scaled by 1/sqrt(DH)
        qc = (q[ra].reshape(GPC, NATOMS, H, DH) * 0.125).astype(np.float32)
        qT = qc.transpose(0, 3, 2, 1).reshape(ROWS, F)
        kT = k[ra].reshape(GPC, NATOMS, H, DH).transpose(0, 3, 2, 1).reshape(ROWS, F)
        # vals 4-seg packing: per 512-edge group put edge ki of segment s at
        # row (group, ki), col s*512; pairs of groups share a [128, 4096] row
        vc = values[re].reshape(EROWS // 512, 4, 128, F)
        vi = vc.transpose(0, 2, 1, 3).reshape(EROWS // 1024, 2, 128, 2048)
        vi = vi.transpose(0, 2, 1, 3).reshape(GPC * 512, 4096)
        in_maps.append({
            "qT": np.ascontiguousarray(qT).astype(bf),
            "kT": np.ascontiguousarray(kT).astype(bf),
            "v": np.ascontiguousarray(v[ra]).reshape(ROWS, F).astype(bf),
            "aw": (np.ascontiguousarray(attn_weights[re]).reshape(ROWS, F)
                   - np.float32(LN32)).astype(bf),
            "vals": np.ascontiguousarray(vi).astype(f8),
        })
    return in_maps


def run_hw(q, k, v, attn_weights, values, **spmd_kwargs):
    in_maps = make_in_maps(q, k, v, attn_weights, values)
    br = run_bass_kernel_spmd(_get_nc(), in_maps, list(range(CORES)),
                              **spmd_kwargs)
    out = np.concatenate(
        [r["out"].astype(np.float32).reshape(ROWS, H, DH) for r in br.results],
        axis=0)
    return out, br


def kernel(q, k, v, attn_weights, values, edges):
    q = np.asarray(q, dtype=np.float32)
    k = np.asarray(k, dtype=np.float32)
    v = np.asarray(v, dtype=np.float32)
    attn_weights = np.asarray(attn_weights, dtype=np.float32)
    values = np.asarray(values, dtype=np.float32)
    e = np.asarray(edges)
    if not _edges_are_dense_blocks(e):
        return _numpy_fallback(q, k, v, attn_weights, values, e)
    return run_hw(q, k, v, attn_weights, values)[0]
